# revision 1
# baseline (speedup 1.0000x reference)
"""Trainium2 Bass kernel for a 3-layer SAGE+GCN GNN on 50k nodes / 800k edges,
sharded across 8 NeuronCores.

Strategy:
  - Nodes are sharded into 8 contiguous ranges; edges assigned to the core that
    owns their dst node (host sorts edges by dst).
  - Per conv, the input features are pre-projected to 128 dims ("table" rows,
    bf16), so every gather moves 256B rows regardless of layer width.
  - Tables for layer 1 are built replicated (each core projects the full x);
    tables for layers 2/3 are built per-shard and exchanged with an AllGather.
  - Aggregation: indirect-DMA gather of each dst-tile's edge rows into SBUF
    (128 edges per partition-chunk), then a one-hot matmul segment-sum on the
    TensorEngine (M[e,d] = (dstlocal[e]==d) * w_e built on VectorE, where w_e
    folds the SAGE mean 1/deg or the GCN edge norm).
  - Everything node-indexed on-chip is kept feature-major ("transposed",
    [feat, node]) so no on-device transposes are ever needed.
"""

import os
import numpy as np
import ml_dtypes

P = 128
NCORES = 8

LAST_EXEC_NS = None
LAST_TRACE = None


# ----------------------------------------------------------------------------
# host-side preprocessing
# ----------------------------------------------------------------------------

REG = int(os.environ.get("GNN_REG", "25000"))  # nodes per dma_gather region (int16 limit)


def _edge_streams(src, dst, w_edge, n, shard, nt):
    """Per-core gather/M-build streams for one edge set, organized for
    nc.gpsimd.dma_gather: per (dst-tile t, src-region r) one gather of
    NV[t][r] valid rows (idx-0 padded to a cross-core-uniform count) plus
    trailing -1 slots up to a chunk multiple.

    Returns per-core (idx16 [128, SIC], dw [128, 2*SK]) plus layout lists.
    """
    nreg = (n + REG - 1) // REG
    percore = []
    counts = np.zeros((NCORES, nt, nreg), np.int64)
    for c in range(NCORES):
        lo, hi = c * shard, (c + 1) * shard
        m = (dst >= lo) & (dst < hi)
        s_c, d_c, w_c = src[m], dst[m] - lo, w_edge[m]
        reg_c = s_c // REG
        order = np.lexsort((reg_c, d_c // P))   # by (tile, region)
        s_c, d_c, w_c, reg_c = s_c[order], d_c[order], w_c[order], reg_c[order]
        key = (d_c // P) * nreg + reg_c
        bounds = np.searchsorted(key, np.arange(nt * nreg + 1))
        counts[c] = (bounds[1:] - bounds[:-1]).reshape(nt, nreg)
        percore.append((s_c, d_c, w_c, bounds))
    NV = counts.max(axis=0)                       # [nt, nreg] max real count
    K = np.maximum((NV + P - 1) // P, 1)          # chunks per (t, r)
    NV = K * P                                    # all slots valid (idx-0 pads)
    Ktot = K.sum(axis=1)                          # chunks per tile
    SK = int(Ktot.sum())
    # column offsets: chunk columns per (t, r); idx16 columns per (t, r)
    coff = np.zeros((nt, nreg), np.int64)
    ioff = np.zeros((nt, nreg), np.int64)
    acc_c = 0
    acc_i = 0
    for t in range(nt):
        for r in range(nreg):
            coff[t, r] = acc_c
            ioff[t, r] = acc_i
            acc_c += K[t, r]
            acc_i += K[t, r] * 8
    SIC = int(acc_i)

    outs = []
    for c in range(NCORES):
        s_c, d_c, w_c, bounds = percore[c]
        idx16 = np.zeros((P, SIC), np.int16)
        dl = np.full((P, SK), -1.0, np.float32)
        wv = np.zeros((P, SK), np.float32)
        for t in range(nt):
            for r in range(nreg):
                b0, b1 = bounds[t * nreg + r], bounds[t * nreg + r + 1]
                cnt = b1 - b0
                kr = int(K[t, r])
                slots = kr * P
                buf_i = np.zeros(slots, np.int32)   # idx-0 pads (always write)
                buf_i[:cnt] = s_c[b0:b1] - r * REG
                buf_d = np.full(slots, -1.0, np.float32)
                buf_d[:cnt] = (d_c[b0:b1] - t * P).astype(np.float32)
                buf_w = np.zeros(slots, np.float32)
                buf_w[:cnt] = w_c[b0:b1]
                # idx16: wrapped over 16 partitions, replicated x8
                cols = kr * 8
                wrap = buf_i.reshape(cols, 16).T.astype(np.int16)  # [16, cols]
                io = int(ioff[t, r])
                idx16[:, io:io + cols] = np.tile(wrap, (8, 1))
                # dl/wv: slot i -> partition i%128, chunk coff+i//128
                co = int(coff[t, r])
                dl[:, co:co + kr] = buf_d.reshape(kr, P).T
                wv[:, co:co + kr] = buf_w.reshape(kr, P).T
        dw = np.concatenate([dl, wv], axis=1).astype(np.float32)
        outs.append((idx16, dw))
    meta_es = dict(K=K.tolist(), NV=NV.tolist(), Ktot=[int(x) for x in Ktot],
                   coff=coff.tolist(), ioff=ioff.tolist(),
                   SK=SK, SIC=SIC, nreg=nreg)
    return outs, meta_es


def _prep(inputs):
    inp = {k: np.asarray(v) for k, v in inputs.items()}
    x = inp["x"].astype(np.float32)
    n, din = x.shape
    assert din == P
    shard = n // NCORES
    nt = (shard + P - 1) // P
    ntx = (n + P - 1) // P
    npad = ntx * P

    src = inp["edge_index"][0].astype(np.int64)
    dst = inp["edge_index"][1].astype(np.int64)
    srca = inp["edge_index_aux"][0].astype(np.int64)
    dsta = inp["edge_index_aux"][1].astype(np.int64)

    deg = np.zeros(n, np.float32)
    np.add.at(deg, dst, 1.0)
    recip_deg = (1.0 / np.maximum(deg, 1.0)).astype(np.float32)
    dega = np.zeros(n, np.float32)
    np.add.at(dega, dsta, 1.0)
    deg_hat = dega + 1.0
    rs = (1.0 / np.sqrt(deg_hat)).astype(np.float32)
    recip_deghat = (1.0 / deg_hat).astype(np.float32)

    sage_streams, es_s = _edge_streams(src, dst, recip_deg[dst], n, shard, nt)
    # GCN self-loop == a (i,i) edge with the same w = rs[dst] form, so fold it
    # into the edge stream (gather row i of the gcn half, scaled by rs[i])
    allnodes = np.arange(n, dtype=np.int64)
    srca_x = np.concatenate([srca, allnodes])
    dsta_x = np.concatenate([dsta, allnodes])
    gcn_streams, es_g = _edge_streams(srca_x, dsta_x, rs[dsta_x], n, shard, nt)

    bf16 = ml_dtypes.bfloat16

    # global transposed x tiles: xt[i*P+f, j] = x[i*P+j, f]
    xpad = np.zeros((npad, P), np.float32)
    xpad[:n] = x
    xt = np.ascontiguousarray(
        xpad.reshape(ntx, P, P).transpose(0, 2, 1).reshape(npad, P)
    ).astype(bf16)

    # packed bf16 weights [P, 2048]
    def w2(a):  # [d,128] -> list of [128,128] tiles
        a = np.asarray(a, np.float32)
        return [a[i * P:(i + 1) * P] for i in range(a.shape[0] // P)]

    wb_tiles = []
    wb_off = {}

    def put_b(name, tiles):
        wb_off[name] = len(wb_tiles) * P
        wb_tiles.extend(tiles)

    put_b("fc1", w2(inp["fc1_W"]))
    for l in (1, 2, 3):
        put_b(f"sWl{l}", w2(inp[f"s{l}_Wl"]))
        put_b(f"gW{l}", w2(inp[f"g{l}_W"]))
        put_b(f"sWr{l}", w2(inp[f"s{l}_Wr"]))
    wb = np.concatenate(wb_tiles, axis=1).astype(bf16)  # [128, 16*128]

    # packed fp32 consts [P, ncols]
    wf_cols = []
    wf_off = {}

    def put_f(name, cols):  # cols: [d] or [d,1] with d multiple-of-128 halves
        a = np.asarray(cols, np.float32).reshape(-1)
        wf_off[name] = len(wf_cols)
        for i in range(a.shape[0] // P):
            wf_cols.append(a[i * P:(i + 1) * P])

    put_f("fc1_b", inp["fc1_b"])
    for l in (1, 2, 3):
        put_f(f"s_bl{l}", inp[f"s{l}_bl"])
        put_f(f"g_b{l}", inp[f"g{l}_b"])
    w_scal = [float(inp[f"w{i}"][0]) for i in range(1, 5)]
    for i in range(1, 5):
        put_f(f"h{i}", inp[f"l{i}_W"].reshape(-1) * w_scal[i - 1])
    wf = np.stack(wf_cols, axis=1).astype(np.float32)  # [128, ncols]
    total_bias = float(sum(float(inp[f"l{i}_b"][0]) * w_scal[i - 1]
                           for i in range(1, 5)))

    iota = np.broadcast_to(np.arange(P, dtype=np.float32), (P, P)).astype(bf16)  # in0 stays bf16
    iota = np.ascontiguousarray(iota)

    # rs for global tiling (gcn table row scale, layer1), pad 1.0
    rs_pad = np.ones(npad, np.float32)
    rs_pad[:n] = rs
    rsg = rs_pad.reshape(ntx, P).T.copy()  # [128, ntx]

    meta = dict(n=n, shard=shard, nt=nt, ntx=ntx, npad=npad,
                es_s=es_s, es_g=es_g,
                wb_off=wb_off, wf_off=wf_off, wf_cols=wf.shape[1],
                total_bias=total_bias)

    in_maps = []
    for c in range(NCORES):
        lo = c * shard
        own = np.zeros((nt * P, P), np.float32)
        nown = min(shard, n - lo)
        ownx = np.zeros((nt * P, P), np.float32)
        ownx[:nown] = x[lo:lo + nown]
        xto = np.ascontiguousarray(
            ownx.reshape(nt, P, P).transpose(0, 2, 1).reshape(nt * P, P)
        ).astype(bf16)
        rso = np.ones(nt * P, np.float32)
        rso[:nown] = rs[lo:lo + nown]
        idx_s, dw_s = sage_streams[c]
        idx_g, dw_g = gcn_streams[c]
        in_maps.append({
            "xt": xt, "xto": xto,
            "idxs": idx_s, "dws": dw_s,
            "idxg": idx_g, "dwg": dw_g,
            "wb": wb, "wf": wf, "iota": iota,
            "rsg": rsg, "rso": rso.reshape(nt, P).T.copy(),
        })
    return meta, in_maps


# ----------------------------------------------------------------------------
# device program
# ----------------------------------------------------------------------------

def _build(meta):
    import concourse.bacc as bacc
    import concourse.bass as bass
    import concourse.mybir as mybir
    import concourse.tile as tile

    dt = mybir.dt
    Alu = mybir.AluOpType
    Act = mybir.ActivationFunctionType

    n, shard, nt, ntx, npad = (meta[k] for k in ("n", "shard", "nt", "ntx", "npad"))
    es_s, es_g = meta["es_s"], meta["es_g"]
    SKs, SKg = es_s["SK"], es_g["SK"]
    SICs, SICg = es_s["SIC"], es_g["SIC"]
    wbo, wfo = meta["wb_off"], meta["wf_off"]

    dbg = bool(int(os.environ.get("GNN_DEBUG", "0")))
    nc = bacc.Bacc("TRN2", target_bir_lowering=False, debug=False,
                   num_devices=NCORES)

    def din(name, shape, dtype):
        return nc.dram_tensor(name, shape, dtype, kind="ExternalInput")

    xt_d = din("xt", [npad, P], dt.bfloat16)
    xto_d = din("xto", [nt * P, P], dt.bfloat16)
    idxs_d = din("idxs", [P, SICs], dt.int16)
    dws_d = din("dws", [P, 2 * SKs], dt.float32)
    idxg_d = din("idxg", [P, SICg], dt.int16)
    dwg_d = din("dwg", [P, 2 * SKg], dt.float32)
    wb_d = din("wb", [P, 16 * P], dt.bfloat16)
    wf_d = din("wf", [P, meta["wf_cols"]], dt.float32)
    iota_d = din("iota", [P, P], dt.bfloat16)
    rsg_d = din("rsg", [P, ntx], dt.float32)
    rso_d = din("rso", [P, nt], dt.float32)
    res_d = nc.dram_tensor("res", [P, nt], dt.float32, kind="ExternalOutput")
    if dbg:
        dbg_tbl1 = nc.dram_tensor("dbg_tbl1", [2 * P, 2 * P], dt.bfloat16,
                                  kind="ExternalOutput")
        dbg_gath = nc.dram_tensor("dbg_gath", [P, es_s["Ktot"][0] * P],
                                  dt.bfloat16, kind="ExternalOutput")
        dbg_agg = nc.dram_tensor("dbg_agg", [P, P], dt.float32,
                                 kind="ExternalOutput")
        dbg_x0 = nc.dram_tensor("dbg_x0", [P, P], dt.float32,
                                kind="ExternalOutput")
        dbg_linr = nc.dram_tensor("dbg_linr", [P, P], dt.float32,
                                  kind="ExternalOutput")

    with tile.TileContext(nc) as tc:
        import contextlib
        _stack = contextlib.ExitStack()
        _ppool = _stack.enter_context(tc.tile_pool(name="persist", bufs=1))
        _dpool = _stack.enter_context(
            tc.tile_pool(name="persistd", bufs=1, space="DRAM"))

        def tc_tile(shape, dtype, space="SBUF", addr_space="Local", name="t"):
            pool = _dpool if space == "DRAM" else _ppool
            return pool.tile(shape, dtype, tag=name, name=name,
                             addr_space=addr_space)

        # --- persistent SBUF ---
        f32, b16 = dt.float32, dt.bfloat16
        x0T = tc_tile([P, nt * P], f32, name="x0T")
        x1aT = tc_tile([P, nt * P], f32, name="x1aT")
        x1bT = tc_tile([P, nt * P], f32, name="x1bT")
        linr = [tc_tile([P, P], f32, name=f"linr{t}") for t in range(nt)]
        resb = tc_tile([P, nt], f32, name="resb")
        wb_s = tc_tile([P, 16 * P], b16, name="wb_s")
        wf_s = tc_tile([P, meta["wf_cols"]], f32, name="wf_s")
        iota_s = tc_tile([P, P], b16, name="iota_s")
        rsg_s = tc_tile([P, ntx], f32, name="rsg_s")
        rso_s = tc_tile([P, nt], f32, name="rso_s")
        dws_s = tc_tile([P, 2 * SKs], f32, name="dws_s")
        dwg_s = tc_tile([P, 2 * SKg], f32, name="dwg_s")

        # --- DRAM tables ---
        tbl1 = tc_tile([npad, 2 * P], b16, space="DRAM", name="tbl1")
        tbl2 = tc_tile([n, 2 * P], b16, space="DRAM", addr_space="Shared",
                       name="tbl2")
        tbl3 = tc_tile([n, 2 * P], b16, space="DRAM", addr_space="Shared",
                       name="tbl3")
        sh2 = tc_tile([shard, 2 * P], b16, space="DRAM", name="sh2")
        sh3 = tc_tile([shard, 2 * P], b16, space="DRAM", name="sh3")

        for t_, d_ in ((wb_s, wb_d), (wf_s, wf_d), (iota_s, iota_d),
                       (rsg_s, rsg_d), (rso_s, rso_d),
                       (dws_s, dws_d), (dwg_s, dwg_d)):
            nc.sync.dma_start(out=t_[:], in_=d_[:])

        with (
            tc.tile_pool(name="xp", bufs=3) as xp,
            tc.tile_pool(name="gp", bufs=3) as gp,
            tc.tile_pool(name="mp", bufs=4) as mp,
            tc.tile_pool(name="op", bufs=4) as op,
            tc.tile_pool(name="bp", bufs=4) as bp,
            tc.tile_pool(name="pp", bufs=2, space="PSUM") as pp,
            tc.tile_pool(name="pq", bufs=4, space="PSUM") as pq,
        ):
            def wbt(name, half=0):  # weight tile [128,128]
                o = wbo[name] + half * P
                return wb_s[:, o:o + P]

            def wfc(name, half=0):  # const col [128,1]
                o = wfo[name] + half
                return wf_s[:, o:o + 1]

            # ---- layer 1: full table (replicated over nodes) ----
            for i in range(ntx):
                xt_t = xp.tile([P, P], b16, tag="xt")
                nc.sync.dma_start(out=xt_t[:], in_=xt_d[i * P:(i + 1) * P, :])
                p1 = pq.tile([P, P], f32, tag="pa")
                nc.tensor.matmul(p1[:], lhsT=wbt("fc1"), rhs=xt_t[:],
                                 start=True, stop=True)
                o1 = bp.tile([P, P], b16, tag="o1")
                nc.scalar.activation(o1[:], p1[:], Act.Relu, bias=wfc("fc1_b"))
                ps = pp.tile([P, P], f32, tag="tbl")
                nc.tensor.matmul(ps[:], lhsT=o1[:], rhs=wbt("sWl1"),
                                 start=True, stop=True)
                pg = pp.tile([P, P], f32, tag="lin")
                nc.tensor.matmul(pg[:], lhsT=o1[:], rhs=wbt("gW1"),
                                 start=True, stop=True)
                tb = bp.tile([P, 2 * P], b16, tag="tb")
                nc.vector.tensor_copy(tb[:, 0:P], ps[:])
                nc.scalar.activation(tb[:, P:2 * P], pg[:], Act.Copy,
                                     scale=rsg_s[:, i:i + 1])
                nc.sync.dma_start(out=tbl1[i * P:(i + 1) * P, :], in_=tb[:])

            # ---- layer 1: own shard (x0T, linr1, gself1, head1) ----
            for t in range(nt):
                sl = slice(t * P, (t + 1) * P)
                xo_t = xp.tile([P, P], b16, tag="xt")
                nc.sync.dma_start(out=xo_t[:], in_=xto_d[t * P:(t + 1) * P, :])
                p1 = pq.tile([P, P], f32, tag="pa")
                nc.tensor.matmul(p1[:], lhsT=wbt("fc1"), rhs=xo_t[:],
                                 start=True, stop=True)
                nc.scalar.activation(x0T[:, sl], p1[:], Act.Relu,
                                     bias=wfc("fc1_b"))
                x0b = bp.tile([P, P], b16, tag="o1")
                nc.vector.tensor_copy(x0b[:], x0T[:, sl])
                plr = pp.tile([P, P], f32, tag="lin")
                nc.tensor.matmul(plr[:], lhsT=wbt("sWr1"), rhs=x0b[:],
                                 start=True, stop=True)
                nc.vector.tensor_scalar(linr[t][:], plr[:], wfc("s_bl1"), None,
                                        op0=Alu.add)
                ph = pp.tile([P, 1], f32, tag="lin")
                nc.tensor.matmul(ph[:], lhsT=x0T[:, sl], rhs=wfc("h1"),
                                 start=True, stop=True)
                nc.vector.tensor_copy(resb[:, t:t + 1], ph[:])

            if dbg:
                nc.sync.dma_start(out=dbg_tbl1[:], in_=tbl1[0:2 * P, :])
                nc.sync.dma_start(out=dbg_x0[:], in_=x0T[:, 0:P])
                nc.sync.dma_start(out=dbg_linr[:], in_=linr[0][:])

            tc.strict_bb_all_engine_barrier()

            # ---- conv layers ----
            def conv_tile(kind, t, tbl, lcur):
                es = es_s if kind == "s" else es_g
                idx_d_ = idxs_d if kind == "s" else idxg_d
                dw = dws_s if kind == "s" else dwg_s
                SK = es["SK"]
                Kt = es["Ktot"][t]
                off = es["coff"][t][0]
                nreg = es["nreg"]
                g = gp.tile([P, Kt * P], b16, tag="gath")
                colofs = 0 if kind == "s" else P
                CAPK = 5  # max chunks (640 rows, HW-proven) per dma_gather
                for r in range(nreg):
                    kr = es["K"][t][r]
                    io = es["ioff"][t][r]
                    co = es["coff"][t][r] - off
                    rlo = r * REG
                    rhi = min(n, (r + 1) * REG)
                    for s in range(0, kr, CAPK):
                        kk = min(CAPK, kr - s)
                        it = mp.tile([P, kk * 8], dt.int16, tag="idxt")
                        nc.sync.dma_start(
                            out=it[:],
                            in_=idx_d_[:, io + s * 8:io + (s + kk) * 8])
                        nc.gpsimd.dma_gather(
                            out_ap=g[:, (co + s) * P:(co + s + kk) * P]
                            .rearrange("p (k e) -> p k e", e=P),
                            in_ap=tbl[rlo:rhi, colofs:colofs + P],
                            idxs_ap=it[:],
                            num_idxs=kk * P,
                            num_idxs_reg=kk * P,
                            elem_size=P,
                            elem_step=2 * P)
                pa = pq.tile([P, P], f32, tag="pa")
                for k in range(Kt):
                    m = mp.tile([P, P], b16, tag="m")
                    nc.vector.tensor_scalar(
                        m[:], iota_s[:],
                        dw[:, off + k:off + k + 1],
                        dw[:, SK + off + k:SK + off + k + 1],
                        op0=Alu.is_equal, op1=Alu.mult)
                    nc.tensor.matmul(pa[:], lhsT=g[:, k * P:(k + 1) * P],
                                     rhs=m[:], start=(k == 0),
                                     stop=(k == Kt - 1))
                if dbg and kind == "s" and t == 0 and lcur == 1:
                    nc.sync.dma_start(out=dbg_gath[:],
                                      in_=g[:, :es_s["Ktot"][0] * P])
                    atmp = op.tile([P, P], f32, tag="atmp")
                    nc.vector.tensor_copy(atmp[:], pa[:])
                    nc.sync.dma_start(out=dbg_agg[:], in_=atmp[:])
                o = op.tile([P, P], f32, tag="c" + kind)
                if kind == "s":
                    nc.vector.tensor_tensor(out=o[:], in0=pa[:],
                                            in1=linr[t][:], op=Alu.add)
                else:
                    nc.vector.tensor_scalar(o[:], pa[:], wfc(f"g_b{lcur}"),
                                            None, op0=Alu.add)
                return o

            for l in (1, 2, 3):
                tbl = (tbl1, tbl2, tbl3)[l - 1]
                sh_next = (sh2, sh3, None)[l - 1]
                tbl_next = (tbl2, tbl3, None)[l - 1]
                for t in range(nt):
                    sl = slice(t * P, (t + 1) * P)
                    oc = conv_tile("s", t, tbl[:], l)
                    oa = conv_tile("g", t, tbl[:], l)
                    if l == 1:
                        nc.vector.tensor_tensor(out=x1aT[:, sl], in0=oc[:],
                                                in1=x0T[:, sl], op=Alu.add)
                        nc.vector.tensor_tensor(out=x1bT[:, sl], in0=oa[:],
                                                in1=x0T[:, sl], op=Alu.add)
                        ocf, oaf = x1aT[:, sl], x1bT[:, sl]
                    else:
                        # += x0 ; += x1 (for out3/out4)
                        nc.vector.tensor_tensor(out=oc[:], in0=oc[:],
                                                in1=x0T[:, sl], op=Alu.add)
                        nc.vector.tensor_tensor(out=oc[:], in0=oc[:],
                                                in1=x1aT[:, sl], op=Alu.add)
                        nc.vector.tensor_tensor(out=oa[:], in0=oa[:],
                                                in1=x0T[:, sl], op=Alu.add)
                        nc.vector.tensor_tensor(out=oa[:], in0=oa[:],
                                                in1=x1bT[:, sl], op=Alu.add)
                        ocf, oaf = oc[:], oa[:]
                    # head on out_{l+1}
                    hname = f"h{l + 1}"
                    ph = pp.tile([P, 1], f32, tag="lin")
                    nc.tensor.matmul(ph[:], lhsT=ocf, rhs=wfc(hname, 0),
                                     start=True, stop=False)
                    nc.tensor.matmul(ph[:], lhsT=oaf, rhs=wfc(hname, 1),
                                     start=False, stop=True)
                    nc.vector.tensor_tensor(out=resb[:, t:t + 1],
                                            in0=resb[:, t:t + 1], in1=ph[:],
                                            op=Alu.add)
                    if l == 3:
                        continue
                    # ---- boundary: tables + linr/gself for layer l+1 ----
                    ocb = bp.tile([P, P], b16, tag="ocb")
                    nc.vector.tensor_copy(ocb[:], ocf)
                    oab = bp.tile([P, P], b16, tag="oab")
                    nc.vector.tensor_copy(oab[:], oaf)
                    ln = l + 1
                    ps = pp.tile([P, P], f32, tag="tbl")
                    nc.tensor.matmul(ps[:], lhsT=ocb[:], rhs=wbt(f"sWl{ln}", 0),
                                     start=True, stop=False)
                    nc.tensor.matmul(ps[:], lhsT=oab[:], rhs=wbt(f"sWl{ln}", 1),
                                     start=False, stop=True)
                    pg = pp.tile([P, P], f32, tag="lin")
                    nc.tensor.matmul(pg[:], lhsT=ocb[:], rhs=wbt(f"gW{ln}", 0),
                                     start=True, stop=False)
                    nc.tensor.matmul(pg[:], lhsT=oab[:], rhs=wbt(f"gW{ln}", 1),
                                     start=False, stop=True)
                    tb = bp.tile([P, 2 * P], b16, tag="tb")
                    nc.vector.tensor_copy(tb[:, 0:P], ps[:])
                    nc.scalar.activation(tb[:, P:2 * P], pg[:], Act.Copy,
                                         scale=rso_s[:, t:t + 1])
                    rt = min(P, shard - t * P)
                    nc.sync.dma_start(out=sh_next[t * P:t * P + rt, :],
                                      in_=tb[:rt, :])
                    plr = pp.tile([P, P], f32, tag="tbl")
                    nc.tensor.matmul(plr[:], lhsT=wbt(f"sWr{ln}", 0), rhs=ocb[:],
                                     start=True, stop=False)
                    nc.tensor.matmul(plr[:], lhsT=wbt(f"sWr{ln}", 1), rhs=oab[:],
                                     start=False, stop=True)
                    nc.vector.tensor_scalar(linr[t][:], plr[:],
                                            wfc(f"s_bl{ln}"), None, op0=Alu.add)
                if l < 3:
                    tc.strict_bb_all_engine_barrier()
                    nc.gpsimd.collective_compute(
                        "AllGather", mybir.AluOpType.bypass,
                        replica_groups=[list(range(NCORES))],
                        ins=[sh_next[:]], outs=[tbl_next[:]])
                    tc.strict_bb_all_engine_barrier()

            # ---- output ----
            nc.vector.tensor_scalar(resb[:], resb[:],
                                    float(meta["total_bias"]), None,
                                    op0=Alu.add)
            nc.sync.dma_start(out=res_d[:], in_=resb[:])
        _stack.close()

    nc.compile()
    return nc


# ----------------------------------------------------------------------------
# entry point
# ----------------------------------------------------------------------------

def _run_and_bench(nc, in_maps, iters):
    """Mirror bass2jax.run_bass_via_pjrt's multi-core path, plus an optional
    pipelined repeat loop to measure marginal per-execution device time."""
    import time
    import jax
    import numpy as np
    from jax.sharding import Mesh, PartitionSpec
    from jax.experimental.shard_map import shard_map
    import concourse.mybir as mybir
    from concourse import bass2jax

    bass2jax.install_neuronx_cc_hook()
    partition_name = (nc.partition_id_tensor.name
                      if nc.partition_id_tensor else None)
    in_names, out_names, out_avals, zero_outs = [], [], [], []
    for alloc in nc.m.functions[0].allocations:
        if not isinstance(alloc, mybir.MemoryLocationSet):
            continue
        name = alloc.memorylocations[0].name
        if alloc.kind == "ExternalInput":
            if name != partition_name:
                in_names.append(name)
        elif alloc.kind == "ExternalOutput":
            shape = tuple(alloc.tensor_shape)
            dtype = mybir.dt.np(alloc.dtype)
            out_names.append(name)
            out_avals.append(jax.core.ShapedArray(shape, dtype))
            zero_outs.append(np.zeros(shape, dtype))
    n_params = len(in_names)
    all_in_names = list(in_names) + out_names
    if partition_name is not None:
        all_in_names.append(partition_name)

    def _body(*args):
        operands = list(args)
        if partition_name is not None:
            operands.append(bass2jax.partition_id_tensor())
        outs = bass2jax._bass_exec_p.bind(
            *operands, out_avals=tuple(out_avals),
            in_names=tuple(all_in_names), out_names=tuple(out_names),
            lowering_input_output_aliases=(),
            sim_require_finite=True, sim_require_nnan=True, nc=nc)
        return tuple(outs)

    devices = jax.devices()[:NCORES]
    mesh = Mesh(np.asarray(devices), ("core",))
    in_specs = (PartitionSpec("core"),) * (n_params + len(out_names))
    out_specs = (PartitionSpec("core"),) * len(out_names)
    sharded = jax.jit(shard_map(_body, mesh=mesh, in_specs=in_specs,
                                out_specs=out_specs, check_rep=False),
                      keep_unused=True)
    concat_in = [
        np.concatenate([np.asarray(in_maps[c][nm]) for c in range(NCORES)], 0)
        for nm in in_names]
    concat_zeros = [np.zeros((NCORES * z.shape[0], *z.shape[1:]), z.dtype)
                    for z in zero_outs]
    out_arrs = sharded(*concat_in, *concat_zeros)
    jax.block_until_ready(out_arrs)

    per_exec_ns = None
    if iters > 0:
        from jax.sharding import NamedSharding
        dev_in = [jax.device_put(a, NamedSharding(mesh, PartitionSpec("core")))
                  for a in concat_in]
        dev_zero = [jax.device_put(z, NamedSharding(mesh, PartitionSpec("core")))
                    for z in concat_zeros]
        r = sharded(*dev_in, *dev_zero)
        jax.block_until_ready(r)
        t1 = time.perf_counter()
        rs = [sharded(*dev_in, *dev_zero) for _ in range(iters)]
        jax.block_until_ready(rs)
        t2 = time.perf_counter()
        per_exec_ns = (t2 - t1) / iters * 1e9

    results = [
        {nm: np.asarray(out_arrs[i]).reshape(NCORES, *out_avals[i].shape)[c]
         for i, nm in enumerate(out_names)}
        for c in range(NCORES)]
    return results, per_exec_ns


def kernel(**inputs):
    global LAST_EXEC_NS, LAST_TRACE

    meta, in_maps = _prep(inputs)
    nc = _build(meta)

    iters = int(os.environ.get("GNN_BENCH", "0"))
    results, per_exec_ns = _run_and_bench(nc, in_maps, iters)
    LAST_EXEC_NS = per_exec_ns
    LAST_TRACE = None

    class _R:
        pass
    res = _R()
    res.results = results

    n, shard, nt = meta["n"], meta["shard"], meta["nt"]
    out = np.empty((n, 1), np.float32)
    for c in range(NCORES):
        r = res.results[c]["res"]  # [128, nt]
        out[c * shard:(c + 1) * shard, 0] = r.T.reshape(-1)[:shard]
    return out



# revision 8
# speedup vs baseline: 1.4114x; 1.4114x over previous
"""Trainium2 Bass kernel for a 3-layer SAGE+GCN GNN on 50k nodes / 800k edges,
sharded across 8 NeuronCores.

Strategy (v2):
  - Nodes sharded into 8 contiguous ranges; edges assigned to the core that
    owns their dst node (host sorts edges by dst tile / src region).
  - Per conv, inputs are pre-projected to 128-dim fp16 "table" rows
    ([node, 256] = sage half | gcn half); every layer's table is built from
    the core's own shard and AllGathered (layer 1 included — no replicated
    full-graph projection).
  - Aggregation: indirect-DMA gather of each dst-tile's edge rows into SBUF,
    then one-hot matmul segment-sum on TensorE (M[e,d] = (dst[e]==d) * w_e
    built on VectorE; w_e folds the SAGE 1/deg or GCN norm).
  - idx16 gather indices are persistent in SBUF (loaded once), removing
    per-gather DMA dispatches.
  - Everything node-indexed on-chip is feature-major ([feat, node]); psum
    evacuations ride the Activation engine; residual adds on VectorE in fp16.
"""

import os
import numpy as np

P = 128
NCORES = 8

LAST_EXEC_NS = None
LAST_TRACE = None


# ----------------------------------------------------------------------------
# host-side preprocessing
# ----------------------------------------------------------------------------

REG = int(os.environ.get("GNN_REG", "25000"))  # nodes per gather region (int16)
CAPK = int(os.environ.get("GNN_CAPK", "5"))    # chunks per dma_gather call


def _edge_streams(src, dst, w_edge, n, shard, nt):
    """Per-core gather/M-build streams for one edge set, organized for
    nc.gpsimd.dma_gather: per (dst-tile t, src-region r) gathers of
    NV[t][r] valid rows (idx-0 padded to a cross-core-uniform count).

    Returns per-core (idx16 [128, SIC], dw [128, 2*SK]) plus layout lists.
    """
    nreg = (n + REG - 1) // REG
    percore = []
    counts = np.zeros((NCORES, nt, nreg), np.int64)
    for c in range(NCORES):
        lo, hi = c * shard, (c + 1) * shard
        m = (dst >= lo) & (dst < hi)
        s_c, d_c, w_c = src[m], dst[m] - lo, w_edge[m]
        reg_c = s_c // REG
        order = np.lexsort((reg_c, d_c // P))   # by (tile, region)
        s_c, d_c, w_c, reg_c = s_c[order], d_c[order], w_c[order], reg_c[order]
        key = (d_c // P) * nreg + reg_c
        bounds = np.searchsorted(key, np.arange(nt * nreg + 1))
        counts[c] = (bounds[1:] - bounds[:-1]).reshape(nt, nreg)
        percore.append((s_c, d_c, w_c, bounds))
    NV = counts.max(axis=0)                       # [nt, nreg] max real count
    K = np.maximum((NV + P - 1) // P, 1)          # chunks per (t, r)
    NV = K * P                                    # all slots valid (idx-0 pads)
    Ktot = K.sum(axis=1)                          # chunks per tile
    SK = int(Ktot.sum())
    coff = np.zeros((nt, nreg), np.int64)
    ioff = np.zeros((nt, nreg), np.int64)
    acc_c = 0
    acc_i = 0
    for t in range(nt):
        for r in range(nreg):
            coff[t, r] = acc_c
            ioff[t, r] = acc_i
            acc_c += K[t, r]
            acc_i += K[t, r] * 8
    SIC = int(acc_i)

    outs = []
    for c in range(NCORES):
        s_c, d_c, w_c, bounds = percore[c]
        idx16 = np.zeros((P, SIC), np.int16)
        dl = np.full((P, SK), -1.0, np.float32)
        wv = np.zeros((P, SK), np.float32)
        for t in range(nt):
            for r in range(nreg):
                b0, b1 = bounds[t * nreg + r], bounds[t * nreg + r + 1]
                cnt = b1 - b0
                kr = int(K[t, r])
                slots = kr * P
                buf_i = np.zeros(slots, np.int32)   # idx-0 pads (always write)
                buf_i[:cnt] = s_c[b0:b1] - r * REG
                buf_d = np.full(slots, -1.0, np.float32)
                buf_d[:cnt] = (d_c[b0:b1] - t * P).astype(np.float32)
                buf_w = np.zeros(slots, np.float32)
                buf_w[:cnt] = w_c[b0:b1]
                # idx16: wrapped over 16 partitions, replicated x8
                cols = kr * 8
                wrap = buf_i.reshape(cols, 16).T.astype(np.int16)  # [16, cols]
                io = int(ioff[t, r])
                idx16[:, io:io + cols] = np.tile(wrap, (8, 1))
                # dl/wv: slot i -> partition i%128, chunk coff+i//128
                co = int(coff[t, r])
                dl[:, co:co + kr] = buf_d.reshape(kr, P).T
                wv[:, co:co + kr] = buf_w.reshape(kr, P).T
        dw = np.concatenate([dl, wv], axis=1).astype(np.float32)
        outs.append((idx16, dw))
    meta_es = dict(K=K.tolist(), NV=NV.tolist(), Ktot=[int(x) for x in Ktot],
                   coff=coff.tolist(), ioff=ioff.tolist(),
                   SK=SK, SIC=SIC, nreg=nreg)
    return outs, meta_es


def _prep(inputs):
    inp = {k: np.asarray(v) for k, v in inputs.items()}
    x = inp["x"].astype(np.float32)
    n, din = x.shape
    assert din == P
    shard = n // NCORES
    nt = (shard + P - 1) // P

    src = inp["edge_index"][0].astype(np.int64)
    dst = inp["edge_index"][1].astype(np.int64)
    srca = inp["edge_index_aux"][0].astype(np.int64)
    dsta = inp["edge_index_aux"][1].astype(np.int64)

    deg = np.zeros(n, np.float32)
    np.add.at(deg, dst, 1.0)
    recip_deg = (1.0 / np.maximum(deg, 1.0)).astype(np.float32)
    dega = np.zeros(n, np.float32)
    np.add.at(dega, dsta, 1.0)
    deg_hat = dega + 1.0
    rs = (1.0 / np.sqrt(deg_hat)).astype(np.float32)

    sage_streams, es_s = _edge_streams(src, dst, recip_deg[dst], n, shard, nt)
    # GCN self-loop == a (i,i) edge with the same w = rs[dst] form, so fold it
    # into the edge stream (gather row i of the gcn half, scaled by rs[i])
    allnodes = np.arange(n, dtype=np.int64)
    srca_x = np.concatenate([srca, allnodes])
    dsta_x = np.concatenate([dsta, allnodes])
    gcn_streams, es_g = _edge_streams(srca_x, dsta_x, rs[dsta_x], n, shard, nt)

    f16 = np.float16

    # packed fp16 weights [P, 16*128]
    def w2(a):  # [d,128] -> list of [128,128] tiles
        a = np.asarray(a, np.float32)
        return [a[i * P:(i + 1) * P] for i in range(a.shape[0] // P)]

    wb_tiles = []
    wb_off = {}

    def put_b(name, tiles):
        wb_off[name] = len(wb_tiles) * P
        wb_tiles.extend(tiles)

    put_b("fc1", w2(inp["fc1_W"]))
    for l in (1, 2, 3):
        put_b(f"sWl{l}", w2(inp[f"s{l}_Wl"]))
        put_b(f"gW{l}", w2(inp[f"g{l}_W"]))
        put_b(f"sWr{l}", w2(inp[f"s{l}_Wr"]))
    wb = np.concatenate(wb_tiles, axis=1).astype(f16)  # [128, 16*128]

    # packed fp32 consts [P, ncols]
    wf_cols = []
    wf_off = {}

    def put_f(name, cols):
        a = np.asarray(cols, np.float32).reshape(-1)
        wf_off[name] = len(wf_cols)
        for i in range(a.shape[0] // P):
            wf_cols.append(a[i * P:(i + 1) * P])

    put_f("fc1_b", inp["fc1_b"])
    for l in (1, 2, 3):
        put_f(f"s_bl{l}", inp[f"s{l}_bl"])
        put_f(f"g_b{l}", inp[f"g{l}_b"])
    wf = np.stack(wf_cols, axis=1).astype(np.float32)  # [128, ncols]

    # fp16 head columns (scaled by w_i): [h1 | h2a h2b | h3a h3b | h4a h4b]
    w_scal = [float(inp[f"w{i}"][0]) for i in range(1, 5)]
    wh_cols = [inp["l1_W"].reshape(-1) * w_scal[0]]
    for i in (2, 3, 4):
        hw_ = inp[f"l{i}_W"].reshape(-1) * w_scal[i - 1]
        wh_cols.append(hw_[:P])
        wh_cols.append(hw_[P:])
    wh = np.stack(wh_cols, axis=1).astype(f16)  # [128, 7]
    total_bias = float(sum(float(inp[f"l{i}_b"][0]) * w_scal[i - 1]
                           for i in range(1, 5)))

    iota = np.broadcast_to(np.arange(P, dtype=np.float32), (P, P)).astype(f16)
    iota = np.ascontiguousarray(iota)

    meta = dict(n=n, shard=shard, nt=nt,
                es_s=es_s, es_g=es_g,
                wb_off=wb_off, wf_off=wf_off, wf_cols=wf.shape[1],
                total_bias=total_bias)

    in_maps = []
    for c in range(NCORES):
        lo = c * shard
        nown = min(shard, n - lo)
        ownx = np.zeros((nt * P, P), np.float32)
        ownx[:nown] = x[lo:lo + nown]
        # feature-major own x: xto[f, t*128+j] = x_own[t*128+j, f]
        xto = np.ascontiguousarray(
            ownx.reshape(nt, P, P).transpose(2, 0, 1).reshape(P, nt * P)
        ).astype(f16)
        rso = np.ones(nt * P, np.float32)
        rso[:nown] = rs[lo:lo + nown]
        idx_s, dw_s = sage_streams[c]
        idx_g, dw_g = gcn_streams[c]
        in_maps.append({
            "xto": xto,
            "idxs": idx_s, "dws": dw_s,
            "idxg": idx_g, "dwg": dw_g,
            "wb": wb, "wf": wf, "wh": wh, "iota": iota,
            "rso": rso.reshape(nt, P).T.copy(),
        })
    return meta, in_maps


# ----------------------------------------------------------------------------
# device program
# ----------------------------------------------------------------------------

def _build(meta):
    import concourse.bacc as bacc
    import concourse.mybir as mybir
    import concourse.tile as tile

    dt = mybir.dt
    Alu = mybir.AluOpType
    Act = mybir.ActivationFunctionType

    n, shard, nt = (meta[k] for k in ("n", "shard", "nt"))
    es_s, es_g = meta["es_s"], meta["es_g"]
    SKs, SKg = es_s["SK"], es_g["SK"]
    SICs, SICg = es_s["SIC"], es_g["SIC"]
    wbo, wfo = meta["wb_off"], meta["wf_off"]

    use_barrier = bool(int(os.environ.get("GNN_BARRIER", "1")))

    nc = bacc.Bacc("TRN2", target_bir_lowering=False, debug=False,
                   num_devices=NCORES)

    def din(name, shape, dtype):
        return nc.dram_tensor(name, shape, dtype, kind="ExternalInput")

    xto_d = din("xto", [P, nt * P], dt.float16)
    idxs_d = din("idxs", [P, SICs], dt.int16)
    dws_d = din("dws", [P, 2 * SKs], dt.float32)
    idxg_d = din("idxg", [P, SICg], dt.int16)
    dwg_d = din("dwg", [P, 2 * SKg], dt.float32)
    wb_d = din("wb", [P, 16 * P], dt.float16)
    wf_d = din("wf", [P, meta["wf_cols"]], dt.float32)
    wh_d = din("wh", [P, 7], dt.float16)
    iota_d = din("iota", [P, P], dt.float16)
    rso_d = din("rso", [P, nt], dt.float32)
    res_d = nc.dram_tensor("res", [P, nt], dt.float32, kind="ExternalOutput")

    with tile.TileContext(nc) as tc:
        import contextlib
        _stack = contextlib.ExitStack()
        _ppool = _stack.enter_context(tc.tile_pool(name="persist", bufs=1))
        _dpool = _stack.enter_context(
            tc.tile_pool(name="persistd", bufs=1, space="DRAM"))

        def tc_tile(shape, dtype, space="SBUF", addr_space="Local", name="t"):
            pool = _dpool if space == "DRAM" else _ppool
            return pool.tile(shape, dtype, tag=name, name=name,
                             addr_space=addr_space)

        f32, f16 = dt.float32, dt.float16
        # --- persistent SBUF ---
        x0T = tc_tile([P, nt * P], f16, name="x0T")
        x1aT = tc_tile([P, nt * P], f16, name="x1aT")
        x1bT = tc_tile([P, nt * P], f16, name="x1bT")
        linr = [tc_tile([P, P], f16, name=f"linr{t}") for t in range(nt)]
        resb = tc_tile([P, nt], f32, name="resb")
        wb_s = tc_tile([P, 16 * P], f16, name="wb_s")
        wf_s = tc_tile([P, meta["wf_cols"]], f32, name="wf_s")
        wh_s = tc_tile([P, 7], f16, name="wh_s")
        iota_s = tc_tile([P, P], f16, name="iota_s")
        rso_s = tc_tile([P, nt], f32, name="rso_s")
        dws_s = tc_tile([P, 2 * SKs], f32, name="dws_s")
        dwg_s = tc_tile([P, 2 * SKg], f32, name="dwg_s")
        idxs_s = tc_tile([P, SICs], dt.int16, name="idxs_s")
        idxg_s = tc_tile([P, SICg], dt.int16, name="idxg_s")

        # --- DRAM tables ---
        sh = [tc_tile([shard, 2 * P], f16, space="DRAM", name=f"sh{l}")
              for l in (1, 2, 3)]
        tbl = [tc_tile([n, 2 * P], f16, space="DRAM", addr_space="Shared",
                       name=f"tbl{l}") for l in (1, 2, 3)]

        for t_, d_ in ((wb_s, wb_d), (wf_s, wf_d), (wh_s, wh_d),
                       (iota_s, iota_d), (rso_s, rso_d),
                       (dws_s, dws_d), (dwg_s, dwg_d),
                       (idxs_s, idxs_d), (idxg_s, idxg_d)):
            nc.sync.dma_start(out=t_[:], in_=d_[:])

        with (
            tc.tile_pool(name="xp", bufs=3) as xp,
            tc.tile_pool(name="gp", bufs=3) as gp,
            tc.tile_pool(name="mp", bufs=6) as mp,
            tc.tile_pool(name="op", bufs=4) as op,
            tc.tile_pool(name="bp", bufs=3) as bp,
            tc.tile_pool(name="pp", bufs=1, space="PSUM") as pp,
            tc.tile_pool(name="pq", bufs=3, space="PSUM") as pq,
            tc.tile_pool(name="pr", bufs=2, space="PSUM") as pr,
        ):
            def wbt(name, half=0):  # weight tile [128,128]
                o = wbo[name] + half * P
                return wb_s[:, o:o + P]

            def wfc(name, half=0):  # const col [128,1]
                o = wfo[name] + half
                return wf_s[:, o:o + 1]

            # group boundaries for batched sh writes: full groups of 4 tiles
            groups = []
            t0 = 0
            while t0 < nt:
                gtiles = min(4, nt - t0)
                if (t0 + gtiles) * P > shard:   # tail tile handled alone
                    gtiles = 1 if gtiles == 1 else gtiles - 1
                groups.append((t0, gtiles))
                t0 += gtiles

            batch_sh = bool(int(os.environ.get("GNN_BATCH_SH", "1")))

            def write_sh(sh_t, t0, gtiles, tb4):
                rows = min(shard - t0 * P, gtiles * P)
                if rows == gtiles * P and batch_sh:
                    o = sh_t[t0 * P:t0 * P + rows, :] \
                        .rearrange("(g p) c -> p g c", p=P)
                    i = tb4[:, :gtiles * 2 * P] \
                        .rearrange("p (g c) -> p g c", c=2 * P)
                    nc.sync.dma_start(out=o, in_=i)
                else:
                    for g in range(gtiles):
                        t = t0 + g
                        rt = min(P, shard - t * P)
                        nc.sync.dma_start(
                            out=sh_t[t * P:t * P + rt, :],
                            in_=tb4[:rt, g * 2 * P:(g + 1) * 2 * P])

            # ---- layer 1 prologue: own-shard table + x0/linr/head ----
            for (t0, gtiles) in groups:
                xo4 = xp.tile([P, 4 * P], f16, tag="xo4")
                nc.sync.dma_start(out=xo4[:, :gtiles * P],
                                  in_=xto_d[:, t0 * P:(t0 + gtiles) * P])
                tb4 = bp.tile([P, 4 * 2 * P], f16, tag="tb4")
                for g in range(gtiles):
                    t = t0 + g
                    sl = slice(t * P, (t + 1) * P)
                    p1 = pq.tile([P, P], f32, tag="pa")
                    nc.tensor.matmul(p1[:], lhsT=wbt("fc1"),
                                     rhs=xo4[:, g * P:(g + 1) * P],
                                     start=True, stop=True)
                    nc.scalar.activation(x0T[:, sl], p1[:], Act.Relu,
                                         bias=wfc("fc1_b"))
                    ps = pp.tile([P, P], f32, tag="tbs")
                    nc.tensor.matmul(ps[:], lhsT=x0T[:, sl], rhs=wbt("sWl1"),
                                     start=True, stop=True)
                    pg = pp.tile([P, P], f32, tag="tbg")
                    nc.tensor.matmul(pg[:], lhsT=x0T[:, sl], rhs=wbt("gW1"),
                                     start=True, stop=True)
                    nc.scalar.activation(tb4[:, g * 2 * P:g * 2 * P + P],
                                         ps[:], Act.Copy)
                    nc.scalar.activation(tb4[:, g * 2 * P + P:(g + 1) * 2 * P],
                                         pg[:], Act.Copy,
                                         scale=rso_s[:, t:t + 1])
                    plr = pp.tile([P, P], f32, tag="plr")
                    nc.tensor.matmul(plr[:], lhsT=wbt("sWr1"), rhs=x0T[:, sl],
                                     start=True, stop=True)
                    nc.vector.tensor_scalar(linr[t][:], plr[:], wfc("s_bl1"),
                                            None, op0=Alu.add)
                    nc.vector.tensor_tensor(out=linr[t][:], in0=linr[t][:],
                                            in1=x0T[:, sl], op=Alu.add)
                    ph = pr.tile([P, 1], f32, tag="ph")
                    nc.tensor.matmul(ph[:], lhsT=x0T[:, sl],
                                     rhs=wh_s[:, 0:1], start=True, stop=True)
                    nc.vector.tensor_copy(resb[:, t:t + 1], ph[:])
                write_sh(sh[0], t0, gtiles, tb4)

            if use_barrier:
                tc.strict_bb_all_engine_barrier()
            nc.gpsimd.collective_compute(
                "AllGather", mybir.AluOpType.bypass,
                replica_groups=[list(range(NCORES))],
                ins=[sh[0][:]], outs=[tbl[0][:]])
            if use_barrier:
                tc.strict_bb_all_engine_barrier()

            # ---- conv layers ----
            def conv_tile(kind, t, tbl_t, lcur):
                es = es_s if kind == "s" else es_g
                idx_sb = idxs_s if kind == "s" else idxg_s
                dw = dws_s if kind == "s" else dwg_s
                SK = es["SK"]
                Kt = es["Ktot"][t]
                off = es["coff"][t][0]
                nreg = es["nreg"]
                g = gp.tile([P, Kt * P], f16, tag="gath" + kind)
                colofs = 0 if kind == "s" else P
                for r in range(nreg):
                    kr = es["K"][t][r]
                    io = es["ioff"][t][r]
                    co = es["coff"][t][r] - off
                    rlo = r * REG
                    rhi = min(n, (r + 1) * REG)
                    for s in range(0, kr, CAPK):
                        kk = min(CAPK, kr - s)
                        nc.gpsimd.dma_gather(
                            out_ap=g[:, (co + s) * P:(co + s + kk) * P]
                            .rearrange("p (k e) -> p k e", e=P),
                            in_ap=tbl_t[rlo:rhi, colofs:colofs + P],
                            idxs_ap=idx_sb[:, io + s * 8:io + (s + kk) * 8],
                            num_idxs=kk * P,
                            num_idxs_reg=kk * P,
                            elem_size=P,
                            elem_step=2 * P)
                pa = pq.tile([P, P], f32, tag="pa")
                for k in range(Kt):
                    m = mp.tile([P, P], f16, tag="m")
                    nc.vector.tensor_scalar(
                        m[:], iota_s[:],
                        dw[:, off + k:off + k + 1],
                        dw[:, SK + off + k:SK + off + k + 1],
                        op0=Alu.is_equal, op1=Alu.mult)
                    nc.tensor.matmul(pa[:], lhsT=g[:, k * P:(k + 1) * P],
                                     rhs=m[:], start=(k == 0),
                                     stop=(k == Kt - 1))
                return pa

            for l in (1, 2, 3):
                tbl_t = tbl[l - 1][:]
                sh_next = sh[l] if l < 3 else None
                for (t0, gtiles) in groups:
                    if l < 3:
                        tb4 = bp.tile([P, 4 * 2 * P], f16, tag="tb4")
                    else:
                        tb4 = None
                    for g_ in range(gtiles):
                        t = t0 + g_
                        sl = slice(t * P, (t + 1) * P)
                        pa = conv_tile("s", t, tbl_t, l)
                        # ocf = agg + linr (linr includes bl + x0 (+x1a))
                        if l == 1:
                            ocf = x1aT[:, sl]
                        else:
                            ocf_t = op.tile([P, P], f16, tag="ocf")
                            ocf = ocf_t[:]
                        nc.vector.tensor_tensor(out=ocf, in0=pa[:],
                                                in1=linr[t][:], op=Alu.add)
                        pg = conv_tile("g", t, tbl_t, l)
                        # oaf = agg + g_b + x0 (+x1b)
                        if l == 1:
                            oaf = x1bT[:, sl]
                        else:
                            oaf_t = op.tile([P, P], f16, tag="oaf")
                            oaf = oaf_t[:]
                        nc.vector.tensor_scalar(oaf, pg[:], wfc(f"g_b{l}"),
                                                None, op0=Alu.add)
                        nc.vector.tensor_tensor(out=oaf, in0=oaf,
                                                in1=x0T[:, sl], op=Alu.add)
                        if l > 1:
                            # x1a is already folded into linr at the boundary;
                            # only the gcn half still needs its x1b residual
                            nc.vector.tensor_tensor(out=oaf, in0=oaf,
                                                    in1=x1bT[:, sl],
                                                    op=Alu.add)
                        # head on out_{l+1}
                        hc = 1 + 2 * (l - 1)
                        ph = pr.tile([P, 1], f32, tag="ph")
                        nc.tensor.matmul(ph[:], lhsT=ocf,
                                         rhs=wh_s[:, hc:hc + 1],
                                         start=True, stop=False)
                        nc.tensor.matmul(ph[:], lhsT=oaf,
                                         rhs=wh_s[:, hc + 1:hc + 2],
                                         start=False, stop=True)
                        nc.vector.tensor_tensor(out=resb[:, t:t + 1],
                                                in0=resb[:, t:t + 1],
                                                in1=ph[:], op=Alu.add)
                        if l == 3:
                            continue
                        # ---- boundary: tables + linr for layer l+1 ----
                        ln = l + 1
                        ps = pp.tile([P, P], f32, tag="tbs")
                        nc.tensor.matmul(ps[:], lhsT=ocf, rhs=wbt(f"sWl{ln}", 0),
                                         start=True, stop=False)
                        nc.tensor.matmul(ps[:], lhsT=oaf, rhs=wbt(f"sWl{ln}", 1),
                                         start=False, stop=True)
                        pgt = pp.tile([P, P], f32, tag="tbg")
                        nc.tensor.matmul(pgt[:], lhsT=ocf, rhs=wbt(f"gW{ln}", 0),
                                         start=True, stop=False)
                        nc.tensor.matmul(pgt[:], lhsT=oaf, rhs=wbt(f"gW{ln}", 1),
                                         start=False, stop=True)
                        nc.scalar.activation(tb4[:, g_ * 2 * P:g_ * 2 * P + P],
                                             ps[:], Act.Copy)
                        nc.scalar.activation(
                            tb4[:, g_ * 2 * P + P:(g_ + 1) * 2 * P],
                            pgt[:], Act.Copy, scale=rso_s[:, t:t + 1])
                        plr = pp.tile([P, P], f32, tag="plr")
                        nc.tensor.matmul(plr[:], lhsT=wbt(f"sWr{ln}", 0),
                                         rhs=ocf, start=True, stop=False)
                        nc.tensor.matmul(plr[:], lhsT=wbt(f"sWr{ln}", 1),
                                         rhs=oaf, start=False, stop=True)
                        nc.vector.tensor_scalar(linr[t][:], plr[:],
                                                wfc(f"s_bl{ln}"), None,
                                                op0=Alu.add)
                        nc.vector.tensor_tensor(out=linr[t][:], in0=linr[t][:],
                                                in1=x0T[:, sl], op=Alu.add)
                        nc.vector.tensor_tensor(out=linr[t][:], in0=linr[t][:],
                                                in1=x1aT[:, sl], op=Alu.add)
                    if l < 3:
                        write_sh(sh_next, t0, gtiles, tb4)
                if l < 3:
                    if use_barrier:
                        tc.strict_bb_all_engine_barrier()
                    nc.gpsimd.collective_compute(
                        "AllGather", mybir.AluOpType.bypass,
                        replica_groups=[list(range(NCORES))],
                        ins=[sh_next[:]], outs=[tbl[l][:]])
                    if use_barrier:
                        tc.strict_bb_all_engine_barrier()

            # ---- output ----
            nc.vector.tensor_scalar(resb[:], resb[:],
                                    float(meta["total_bias"]), None,
                                    op0=Alu.add)
            nc.sync.dma_start(out=res_d[:], in_=resb[:])
        _stack.close()

    nc.compile()
    return nc


# ----------------------------------------------------------------------------
# entry point
# ----------------------------------------------------------------------------

def _run_and_bench(nc, in_maps, iters):
    """Mirror bass2jax.run_bass_via_pjrt's multi-core path, plus an optional
    pipelined repeat loop to measure marginal per-execution device time."""
    import time
    import jax
    import numpy as np
    from jax.sharding import Mesh, PartitionSpec
    from jax.experimental.shard_map import shard_map
    import concourse.mybir as mybir
    from concourse import bass2jax

    bass2jax.install_neuronx_cc_hook()
    partition_name = (nc.partition_id_tensor.name
                      if nc.partition_id_tensor else None)
    in_names, out_names, out_avals, zero_outs = [], [], [], []
    for alloc in nc.m.functions[0].allocations:
        if not isinstance(alloc, mybir.MemoryLocationSet):
            continue
        name = alloc.memorylocations[0].name
        if alloc.kind == "ExternalInput":
            if name != partition_name:
                in_names.append(name)
        elif alloc.kind == "ExternalOutput":
            shape = tuple(alloc.tensor_shape)
            dtype = mybir.dt.np(alloc.dtype)
            out_names.append(name)
            out_avals.append(jax.core.ShapedArray(shape, dtype))
            zero_outs.append(np.zeros(shape, dtype))
    n_params = len(in_names)
    all_in_names = list(in_names) + out_names
    if partition_name is not None:
        all_in_names.append(partition_name)

    def _body(*args):
        operands = list(args)
        if partition_name is not None:
            operands.append(bass2jax.partition_id_tensor())
        outs = bass2jax._bass_exec_p.bind(
            *operands, out_avals=tuple(out_avals),
            in_names=tuple(all_in_names), out_names=tuple(out_names),
            lowering_input_output_aliases=(),
            sim_require_finite=True, sim_require_nnan=True, nc=nc)
        return tuple(outs)

    devices = jax.devices()[:NCORES]
    mesh = Mesh(np.asarray(devices), ("core",))
    in_specs = (PartitionSpec("core"),) * (n_params + len(out_names))
    out_specs = (PartitionSpec("core"),) * len(out_names)
    sharded = jax.jit(shard_map(_body, mesh=mesh, in_specs=in_specs,
                                out_specs=out_specs, check_rep=False),
                      keep_unused=True)
    concat_in = [
        np.concatenate([np.asarray(in_maps[c][nm]) for c in range(NCORES)], 0)
        for nm in in_names]
    concat_zeros = [np.zeros((NCORES * z.shape[0], *z.shape[1:]), z.dtype)
                    for z in zero_outs]
    out_arrs = sharded(*concat_in, *concat_zeros)
    jax.block_until_ready(out_arrs)

    per_exec_ns = None
    if iters > 0:
        from jax.sharding import NamedSharding
        dev_in = [jax.device_put(a, NamedSharding(mesh, PartitionSpec("core")))
                  for a in concat_in]
        dev_zero = [jax.device_put(z, NamedSharding(mesh, PartitionSpec("core")))
                    for z in concat_zeros]
        r = sharded(*dev_in, *dev_zero)
        jax.block_until_ready(r)
        t1 = time.perf_counter()
        rs = [sharded(*dev_in, *dev_zero) for _ in range(iters)]
        jax.block_until_ready(rs)
        t2 = time.perf_counter()
        per_exec_ns = (t2 - t1) / iters * 1e9

    results = [
        {nm: np.asarray(out_arrs[i]).reshape(NCORES, *out_avals[i].shape)[c]
         for i, nm in enumerate(out_names)}
        for c in range(NCORES)]
    return results, per_exec_ns


def kernel(**inputs):
    global LAST_EXEC_NS, LAST_TRACE

    meta, in_maps = _prep(inputs)
    nc = _build(meta)

    iters = int(os.environ.get("GNN_BENCH", "0"))
    results, per_exec_ns = _run_and_bench(nc, in_maps, iters)
    LAST_EXEC_NS = per_exec_ns
    LAST_TRACE = None

    n, shard, nt = meta["n"], meta["shard"], meta["nt"]
    out = np.empty((n, 1), np.float32)
    for c in range(NCORES):
        r = results[c]["res"]  # [128, nt]
        out[c * shard:(c + 1) * shard, 0] = r.T.reshape(-1)[:shard]
    return out
